# revision 1
# baseline (speedup 1.0000x reference)
"""Multi-head causal attention with RoPE on 8 Trainium2 NeuronCores.

Sharding: tensor-parallel over heads. Each core owns 2 of the 16 heads
(a 128-wide slice of D_OUT): it computes Q/K/V projections for its slice
(column-sliced Wq/Wk/Wv), RoPE, causal attention, and a row-sliced
out-projection partial. The 8 partials are summed on the host (the
all-reduce of the row-parallel out projection) and the bias added once.

All matmul operands are fp16 (weights/x quantized on the host, Q/K/V/P
requantized on-chip); PSUM accumulation stays fp32, so end-to-end error
is ~4e-4. fp16 matters on real HW: f32r matmuls use a slow fused 4-byte
self-loading weight path (~4x slower than the cost model's optimistic
rate), while fp16 streams 1 col/cycle with overlapped FWL weight loads.

Scheduling: one flattened software pipeline across all reps/batches/
windows for the in-order engine queues. Per key-block the two heads'
scores live in one 2-bank PSUM tile so a single wide exp serves both
(halves ACT instruction overhead). The next window's projections, its
RoPE, and the PREVIOUS window's out-projection units are interleaved
into the attention loop, so PE never sits behind the ACT exp chain or
the normalize chain. During each batch's last (longest) window the
pipeline preps two windows ahead so the short w0 window isn't asked to
hide a full projection. Normalize is a pure-DVE chain (recip with
cross-partition addressing, zero-mask stream_shuffle quadrant
broadcasts, half-plane product writes) so no cross-engine semaphore
hop sits between the last PV and releasing the ctx PSUM banks; a small
reserve of interleave quanta is flushed after it so PE stays fed while
the chain drains.
"""

import sys

sys.path.insert(0, "/opt/trn_rl_repo")

from contextlib import ExitStack

import numpy as np

import concourse.bass as bass
import concourse.tile as tile
from concourse import bacc, mybir
from concourse.bass import MemorySpace
from concourse.bass_utils import run_bass_kernel_spmd

B, T, D, H, DH = 2, 2048, 1024, 16, 64
NCORES = 8
DC = D // NCORES  # 128: d-slice per core (2 heads)
QSB = 512  # query superblock
NW = T // QSB  # windows per batch

f32 = mybir.dt.float32
f32r = mybir.dt.float32r
f16 = mybir.dt.float16
AF = mybir.ActivationFunctionType

SWAP_MASK = []
for _i in range(16):
    SWAP_MASK += [2 * _i + 1, 2 * _i]

_CACHE = {}


def _build(reps=1, pipeline=True, delay_out=True, merge_out=False, cp="split", nrm="dve", qt_bufs=3, rope_bufs=12, s_bufs=2, ctx_bufs=2, ost_bufs=8, p_bufs=6, proj_bufs=1, rsv=2):
    nc = bacc.Bacc("TRN2", target_bir_lowering=False, debug=False)
    xT = nc.dram_tensor("xt", [B, D, T], f16, kind="ExternalInput").ap()
    wq = nc.dram_tensor("wq", [D, DC], f16, kind="ExternalInput").ap()
    wk = nc.dram_tensor("wk", [D, DC], f16, kind="ExternalInput").ap()
    wv = nc.dram_tensor("wv", [D, DC], f16, kind="ExternalInput").ap()
    wo = nc.dram_tensor("wo", [DC, D], f16, kind="ExternalInput").ap()
    ropec = nc.dram_tensor("ropec", [DC, T], f32, kind="ExternalInput").ap()
    ropes = nc.dram_tensor("ropes", [DC, T], f32, kind="ExternalInput").ap()
    mneg = nc.dram_tensor("mneg", [128, 128], f32, kind="ExternalInput").ap()
    tril = nc.dram_tensor("tril", [128, 2, 128], f16, kind="ExternalInput").ap()
    ident = nc.dram_tensor("ident", [128, 128], f16, kind="ExternalInput").ap()
    vones = nc.dram_tensor("vones", [128, 16], f16, kind="ExternalInput").ap()
    out = nc.dram_tensor("out", [B, T, D], f16, kind="ExternalOutput").ap()

    with tile.TileContext(nc) as tc, ExitStack() as ctx:
        const = ctx.enter_context(tc.tile_pool(name="const", bufs=1))
        x_pool = ctx.enter_context(tc.tile_pool(name="x_pool", bufs=4))
        kt_pool = ctx.enter_context(tc.tile_pool(name="kt_pool", bufs=2))
        va_pool = ctx.enter_context(tc.tile_pool(name="va_pool", bufs=2))
        qt_pool = ctx.enter_context(tc.tile_pool(name="qt_pool", bufs=qt_bufs))
        rope_pool = ctx.enter_context(tc.tile_pool(name="rope_pool", bufs=rope_bufs))
        vt_pool = ctx.enter_context(tc.tile_pool(name="vt_pool", bufs=2))
        p_pool = ctx.enter_context(tc.tile_pool(name="p_pool", bufs=p_bufs))
        nrm_pool = ctx.enter_context(tc.tile_pool(name="nrm_pool", bufs=2))
        ctxn_pool = ctx.enter_context(tc.tile_pool(name="ctxn_pool", bufs=2))
        ost_pool = ctx.enter_context(tc.tile_pool(name="ost_pool", bufs=ost_bufs))

        proj_ps = ctx.enter_context(
            tc.tile_pool(name="proj_ps", bufs=proj_bufs, space=MemorySpace.PSUM)
        )
        s_ps = ctx.enter_context(
            tc.tile_pool(name="s_ps", bufs=s_bufs, space=MemorySpace.PSUM)
        )
        ctx_ps = ctx.enter_context(
            tc.tile_pool(name="ctx_ps", bufs=ctx_bufs, space=MemorySpace.PSUM)
        )
        out_ps = ctx.enter_context(
            tc.tile_pool(name="out_ps", bufs=1, space=MemorySpace.PSUM)
        )

        # ---- constants (first-use order; first window's loads split) ----
        wq_sb = const.tile([128, 8, DC], f16)
        wk_sb = const.tile([128, 8, DC], f16)
        wv_sb = const.tile([128, 8, DC], f16)
        wqr = wq.rearrange("(c p) m -> p c m", p=128)
        xw0 = x_pool.tile([128, 8, QSB], f16, name="xw0", tag="xw")
        xr0 = xT[0, :, 0:QSB].rearrange("(c p) t -> p c t", p=128)
        for kc in range(8):
            nc.sync.dma_start(out=wq_sb[:, kc : kc + 1], in_=wqr[:, kc : kc + 1])
            nc.sync.dma_start(out=xw0[:, kc : kc + 1], in_=xr0[:, kc : kc + 1])
        nc.sync.dma_start(out=wk_sb, in_=wk.rearrange("(c p) m -> p c m", p=128))
        ct_sb = const.tile([128, T], f32)
        st_sb = const.tile([128, T], f32)
        nc.sync.dma_start(out=ct_sb, in_=ropec)
        nc.sync.dma_start(out=st_sb, in_=ropes)
        nc.sync.dma_start(out=wv_sb, in_=wv.rearrange("(c p) m -> p c m", p=128))
        ident_sb = const.tile([128, 128], f16)
        nc.sync.dma_start(out=ident_sb, in_=ident)
        tril_sb = const.tile([128, 2, 128], f16)
        nc.sync.dma_start(out=tril_sb, in_=tril)
        wo_sb = const.tile([128, D], f16)
        nc.sync.dma_start(out=wo_sb, in_=wo)
        # persistent recip staging tile: lanes 1-31 are never read by the
        # zero-mask shuffles but must be initialized once for the checkers
        rc32a = const.tile([32, QSB], f32)
        rc32b = const.tile([32, QSB], f32)
        rc32s = [rc32a, rc32b]
        nc.gpsimd.memset(rc32a, 0.0)
        nc.gpsimd.memset(rc32b, 0.0)

        if True:  # single flattened pipeline across reps
            KTs, VAs = {}, {}

            def prep_window(b, w, first):
                """Emit xw DMA now; return (qt_tile, quanta closures)."""
                t0 = w * QSB
                if w == 0:
                    KTs[b] = kt_pool.tile([128, T], f16, name=f"KT{b}", tag="KT")
                    VAs[b] = va_pool.tile(
                        [128, 16, 130], f16, name=f"VA{b}", tag="VA"
                    )
                    nc.sync.dma_start(
                        out=VAs[b][:, :, 64:65],
                        in_=vones.rearrange("p (c o) -> p c o", o=1),
                    )
                    nc.sync.dma_start(
                        out=VAs[b][:, :, 129:130],
                        in_=vones.rearrange("p (c o) -> p c o", o=1),
                    )
                KT, VA = KTs[b], VAs[b]
                if first:
                    xw = xw0
                else:
                    xw = x_pool.tile([128, 8, QSB], f16, name="xw", tag="xw")
                    nc.sync.dma_start(
                        out=xw,
                        in_=xT[b, :, t0 : t0 + QSB].rearrange(
                            "(c p) t -> p c t", p=128
                        ),
                    )
                qt = qt_pool.tile([128, QSB], f16, name="qt")
                state = {}

                def mk_proj(wsb, key):
                    def run():
                        ps = proj_ps.tile([128, QSB], f32, tag="proj", name="ps")
                        for kc in range(8):
                            nc.tensor.matmul(
                                ps,
                                wsb[:, kc],
                                xw[:, kc],
                                start=(kc == 0),
                                stop=(kc == 7),
                            )
                        state[key] = ps

                    return run

                def mk_rope(key, dest_fn):
                    def run():
                        ps = state.pop(key)
                        dest = dest_fn()
                        sh = rope_pool.tile([128, QSB], f32, tag="ropet", name="sh")
                        nc.vector.stream_shuffle(sh, ps, SWAP_MASK)
                        m1 = rope_pool.tile([128, QSB], f16, tag="ropet", name="m1")
                        nc.vector.tensor_mul(m1, ps, ct_sb[:, t0 : t0 + QSB])
                        m2 = rope_pool.tile([128, QSB], f16, tag="ropet", name="m2")
                        nc.gpsimd.tensor_mul(m2, sh, st_sb[:, t0 : t0 + QSB])
                        nc.gpsimd.tensor_add(dest, m1, m2)

                    return run

                def mk_vpath():
                    def run():
                        ps = state.pop("v")
                        vt = vt_pool.tile([128, QSB], f16, name="vt")
                        if cp in ("dve", "vtdve"):
                            nc.vector.tensor_copy(vt, ps)
                        else:
                            nc.scalar.copy(vt, ps)
                        vblk = proj_ps.tile([128, 4, 128], f16, tag="proj", name="vb")
                        for i in range(4):
                            nc.tensor.transpose(
                                vblk[:, i], vt[:, 128 * i : 128 * i + 128], ident_sb
                            )
                        for i in range(4):
                            tb = 4 * w + i
                            on_dve = cp == "dve" or i % 2 == 0
                            fn = nc.vector.tensor_copy if on_dve else nc.scalar.copy
                            fn(VA[:, tb, 0:64], vblk[:, i, 0:64])
                            fn(VA[:, tb, 65:129], vblk[:, i, 64:128])

                    return run

                quanta = [
                    mk_proj(wq_sb, "q"),
                    mk_rope("q", lambda: qt),
                    mk_proj(wk_sb, "k"),
                    mk_rope("k", lambda: KT[:, t0 : t0 + QSB]),
                    mk_proj(wv_sb, "v"),
                    mk_vpath(),
                ]
                return qt, quanta

            def outproj_quanta(ctxn, b, t0):
                """One closure per (ts, eh) out-proj unit, for interleaving."""
                qs = []
                for ts in range(4):
                    for eh in range(2):

                        def run(ts=ts, eh=eh):
                            ops = out_ps.tile([128, 512], f32, name="ops", tag="ops")
                            nc.tensor.matmul(
                                ops,
                                ctxn[:, 128 * ts : 128 * ts + 128],
                                wo_sb[:, 512 * eh : 512 * eh + 512],
                                start=True,
                                stop=True,
                            )
                            ost = ost_pool.tile([128, 512], f16, name="ost", tag="ost")
                            if eh == 0 or cp in ("dve", "ostdve"):
                                nc.vector.tensor_copy(ost, ops)
                            else:
                                nc.scalar.copy(ost, ops)
                            nc.sync.dma_start(
                                out=out[
                                    b,
                                    t0 + 128 * ts : t0 + 128 * ts + 128,
                                    512 * eh : 512 * eh + 512,
                                ],
                                in_=ost,
                            )

                        qs.append(run)
                return qs

            def emit_outproj(ctxn, b, t0):
                for ts in range(4):
                    if merge_out:
                        ops = out_ps.tile([128, 2, 512], f32, name="ops", tag="ops")
                        for eh in range(2):
                            nc.tensor.matmul(
                                ops[:, eh],
                                ctxn[:, 128 * ts : 128 * ts + 128],
                                wo_sb[:, 512 * eh : 512 * eh + 512],
                                start=True,
                                stop=True,
                            )
                        ost = ost_pool.tile([128, 2, 512], f16, name="ost", tag="ost")
                        if ts % 2 == 0:
                            nc.vector.tensor_copy(ost, ops)
                        else:
                            nc.scalar.copy(ost, ops)
                        nc.sync.dma_start(
                            out=out[b, t0 + 128 * ts : t0 + 128 * ts + 128, :],
                            in_=ost.rearrange("p a n -> p (a n)"),
                        )
                    else:
                        for eh in range(2):
                            ops = out_ps.tile([128, 512], f32, name="ops", tag="ops")
                            nc.tensor.matmul(
                                ops,
                                ctxn[:, 128 * ts : 128 * ts + 128],
                                wo_sb[:, 512 * eh : 512 * eh + 512],
                                start=True,
                                stop=True,
                            )
                            ost = ost_pool.tile([128, 512], f16, name="ost", tag="ost")
                            if eh == 0 or cp in ("dve", "ostdve"):
                                nc.vector.tensor_copy(ost, ops)
                            else:
                                nc.scalar.copy(ost, ops)
                            nc.sync.dma_start(
                                out=out[
                                    b,
                                    t0 + 128 * ts : t0 + 128 * ts + 128,
                                    512 * eh : 512 * eh + 512,
                                ],
                                in_=ost,
                            )

            seq = [
                (b, w) for rep in range(reps) for b in range(B) for w in range(NW)
            ]
            qts = {}
            qt0, quanta0 = prep_window(0, 0, first=True)
            for q in quanta0:
                q()
            qts[0] = qt0
            pending = None

            for i, (b, w) in enumerate(seq):
                t0 = w * QSB
                KT, VA = KTs[b], VAs[b]
                qt = qts.pop(i)

                # out-projection of the PREVIOUS window: its deps are long
                # satisfied, so spread its MM+copy units into the interleave
                # (fills out_ps MM->copy->MM serialization with attention work)
                oq = []
                if delay_out and pending is not None:
                    oq = outproj_quanta(*pending)
                    pending = None
                # prefetch: always have window i+1 prepped; during the longest
                # window of each batch (w == NW-1) also prep i+2 so the
                # following SHORT w0 window isn't asked to hide a full prep
                pq = []
                preps = [i + 1, i + 2] if w == NW - 1 else [i + 1]
                for j in preps:
                    if j < len(seq) and j not in qts:
                        jb, jw = seq[j]
                        qtj, pj = prep_window(jb, jw, first=False)
                        qts[j] = qtj
                        pq += pj
                nq = oq + pq
                if not pipeline:
                    for q in nq:
                        q()
                    nq = []

                nkb = 4 * w + 4
                cps = [
                    ctx_ps.tile([65, QSB], f32, tag="ctx", name=f"cps{h}")
                    for h in range(2)
                ]
                emitted = 0
                for kb in range(nkb):
                    o = kb - 4 * w
                    col0 = 128 * o if o > 0 else 0
                    ncols = QSB - col0
                    sps = s_ps.tile([128, 2, QSB], f32, tag="s", name="sps")
                    for h in range(2):
                        nc.tensor.matmul(
                            sps[:, h, :ncols],
                            KT[64 * h : 64 * h + 64, 128 * kb : 128 * kb + 128],
                            qt[64 * h : 64 * h + 64, col0:QSB],
                            start=True,
                            stop=True,
                        )
                    pt = p_pool.tile([128, 2, QSB], f16, tag="pt", name="pt")
                    nc.scalar.activation(
                        pt[:, :, :ncols], sps[:, :, :ncols], AF.Exp, scale=0.125
                    )
                    if o >= 0:
                        # causal mask as a 0/1 multiply on the idle Pool
                        # engine, post-exp in SBUF fp16 — keeps the mask off
                        # the DVE queue and off the QK->exp critical path
                        nc.gpsimd.tensor_mul(
                            pt[:, :, 0:128], pt[:, :, 0:128], tril_sb
                        )
                    # interleave next-window projection quanta while ACT runs exp
                    want = (kb + 1) * len(nq) // nkb
                    while emitted < want:
                        nq[emitted]()
                        emitted += 1
                    for h in range(2):
                        nc.tensor.matmul(
                            cps[h][:, col0:QSB],
                            VA[:, kb, 65 * h : 65 * h + 65],
                            pt[:, h, :ncols],
                            start=(kb == 0),
                            stop=(kb == nkb - 1),
                        )
                # flush all but a small reserve of quanta; the reserve is
                # emitted after normalize so PE has work queued while the
                # recip/broadcast/mul chain frees the ctx PSUM banks
                reserve = min(rsv, len(nq) - emitted)
                while emitted < len(nq) - reserve:
                    nq[emitted]()
                    emitted += 1

                # normalize this window (emitted before the reserve so DVE
                # starts the recip as soon as the last PV lands)
                ctxn = ctxn_pool.tile([128, QSB], f16, name="ctxn")
                for h in range(2):
                    if nrm == "copy":  # timing probe only: skip the divide
                        nc.vector.tensor_copy(
                            ctxn[64 * h : 64 * h + 64, :], cps[h][0:64, :]
                        )
                        continue
                    bc = nrm_pool.tile([64, QSB], f32, tag="bc", name="bc")
                    if nrm == "dve":
                        # all-DVE chain: recip into lane 0 of the persistent
                        # staging tile, then zero-mask shuffles broadcast it
                        # to both quadrants (no cross-engine semaphore hops)
                        rc32 = rc32s[h]
                        nc.vector.reciprocal(rc32[0:1, :], cps[h][64:65, :])
                        nc.vector.stream_shuffle(bc[0:32, :], rc32, [0] * 32)
                        nc.vector.stream_shuffle(bc[32:64, :], rc32, [0] * 32)
                    else:
                        rc0 = nrm_pool.tile([1, QSB], f32, tag="rc0", name="rc0")
                        nc.vector.reciprocal(rc0, cps[h][64:65, :])
                        nc.gpsimd.partition_broadcast(bc, rc0, channels=64)
                    # DVE bank routing at nch=64 can write either half-plane,
                    # so head1's product lands at partitions 64-127 directly
                    nc.vector.tensor_mul(
                        ctxn[64 * h : 64 * h + 64, :], cps[h][0:64, :], bc
                    )
                while emitted < len(nq):
                    nq[emitted]()
                    emitted += 1
                if delay_out:
                    pending = (ctxn, b, t0)
                else:
                    emit_outproj(ctxn, b, t0)

            if pending is not None:
                emit_outproj(*pending)

    nc.compile()
    return nc


def _host_inputs(x, Wq, Wk, Wv, Wo):
    xT = np.ascontiguousarray(x.transpose(0, 2, 1)).astype(np.float16)

    pos = np.arange(T, dtype=np.float64)
    inv_freq = np.power(10000.0, -2.0 * np.arange(0, DH, 2) / DH)  # (32,)
    freqs = pos[:, None] * inv_freq[None, :]  # (T, 32)
    cos = np.cos(freqs)
    sin = np.sin(freqs)
    ct = np.empty((DC, T), np.float32)
    st = np.empty((DC, T), np.float32)
    for p in range(DC):
        i = (p % DH) // 2
        ct[p] = cos[:, i]
        st[p] = sin[:, i] * (-1.0 if p % 2 == 0 else 1.0)

    pp, cc = np.meshgrid(np.arange(128), np.arange(128), indexing="ij")
    mneg = np.where(pp <= cc, 0.0, -1e9).astype(np.float32)
    tril2 = np.broadcast_to(
        np.where(pp <= cc, 1.0, 0.0).astype(np.float16)[:, None, :], (128, 2, 128)
    ).copy()
    ident = np.eye(128, dtype=np.float32)

    per_core = []
    for c in range(NCORES):
        sl = slice(c * DC, (c + 1) * DC)
        per_core.append(
            {
                "xt": xT,
                "wq": np.ascontiguousarray(Wq[:, sl]).astype(np.float16),
                "wk": np.ascontiguousarray(Wk[:, sl]).astype(np.float16),
                "wv": np.ascontiguousarray(Wv[:, sl]).astype(np.float16),
                "wo": np.ascontiguousarray(Wo[sl, :]).astype(np.float16),
                "ropec": ct,
                "ropes": st,
                "mneg": mneg,
                "tril": tril2,
                "ident": ident.astype(np.float16),
                "vones": np.ones((128, 16), np.float16),
            }
        )
    return per_core


def kernel(x, Wq, Wk, Wv, Wo, bo):
    x = np.asarray(x, np.float32)
    Wq = np.asarray(Wq, np.float32)
    Wk = np.asarray(Wk, np.float32)
    Wv = np.asarray(Wv, np.float32)
    Wo = np.asarray(Wo, np.float32)
    bo = np.asarray(bo, np.float32)

    if "nc" not in _CACHE:
        _CACHE["nc"] = _build()
    nc = _CACHE["nc"]

    in_maps = _host_inputs(x, Wq, Wk, Wv, Wo)
    res = run_bass_kernel_spmd(nc, in_maps, list(range(NCORES)))
    acc = res.results[0]["out"].astype(np.float64)
    for c in range(1, NCORES):
        acc += res.results[c]["out"]
    acc += bo.astype(np.float64)
    return acc.astype(np.float32)



# revision 56
# speedup vs baseline: 1.0734x; 1.0734x over previous
"""Multi-head causal attention with RoPE on 8 Trainium2 NeuronCores.

Sharding: tensor-parallel over heads. Each core owns 2 of the 16 heads
(a 128-wide slice of D_OUT): it computes Q/K/V projections for its slice
(column-sliced Wq/Wk/Wv), RoPE, causal attention, and a row-sliced
out-projection partial. The 8 partials are summed on the host (the
all-reduce of the row-parallel out projection) and the bias added once.

All matmul operands are fp16 (weights/x quantized on the host, Q/K/V/P
requantized on-chip); PSUM accumulation stays fp32, so end-to-end error
is ~4e-4. fp16 matters on real HW: f32r matmuls use a slow fused 4-byte
self-loading weight path (~4x slower than the cost model's optimistic
rate), while fp16 streams 1 col/cycle with overlapped FWL weight loads.

Scheduling: one flattened software pipeline across all reps/batches/
windows for the in-order engine queues. Per key-block the two heads'
scores live in one 2-bank PSUM tile so a single wide exp serves both
(halves ACT instruction overhead). The next window's projections, its
RoPE, and the PREVIOUS window's out-projection units are interleaved
into the attention loop, so PE never sits behind the ACT exp chain or
the normalize chain. During each batch's last (longest) window the
pipeline preps two windows ahead so the short w0 window isn't asked to
hide a full projection. Normalize is a pure-DVE chain (recip with
cross-partition addressing, zero-mask stream_shuffle quadrant
broadcasts, half-plane product writes) so no cross-engine semaphore
hop sits between the last PV and releasing the ctx PSUM banks; a small
reserve of interleave quanta is flushed after it so PE stays fed while
the chain drains.
"""

import sys

sys.path.insert(0, "/opt/trn_rl_repo")

from contextlib import ExitStack

import numpy as np

import concourse.bass as bass
import concourse.tile as tile
from concourse import bacc, mybir
from concourse.bass import MemorySpace
from concourse.bass_utils import run_bass_kernel_spmd

B, T, D, H, DH = 2, 2048, 1024, 16, 64
NCORES = 8
DC = D // NCORES  # 128: d-slice per core (2 heads)
QSB = 512  # query superblock
NW = T // QSB  # windows per batch

f32 = mybir.dt.float32
f32r = mybir.dt.float32r
f16 = mybir.dt.float16
AF = mybir.ActivationFunctionType

SWAP_MASK = []
for _i in range(16):
    SWAP_MASK += [2 * _i + 1, 2 * _i]

_CACHE = {}


def _build(reps=1, pipeline=True, delay_out=True, merge_out=False, cp="split", nrm="gps", qt_bufs=3, rope_bufs=12, s_bufs=3, ctx_bufs=2, ost_bufs=8, p_bufs=6, proj_bufs=2, rsv=6, skew=0, vxp="pe", mask="mm"):
    nc = bacc.Bacc("TRN2", target_bir_lowering=False, debug=False)
    xT = nc.dram_tensor("xt", [B, D, T], f16, kind="ExternalInput").ap()
    wq = nc.dram_tensor("wq", [D, DC], f16, kind="ExternalInput").ap()
    wk = nc.dram_tensor("wk", [D, DC], f16, kind="ExternalInput").ap()
    wv = nc.dram_tensor("wv", [D, DC], f16, kind="ExternalInput").ap()
    wo = nc.dram_tensor("wo", [DC, D], f16, kind="ExternalInput").ap()
    ropec = nc.dram_tensor("ropec", [DC, T], f32, kind="ExternalInput").ap()
    ropes = nc.dram_tensor("ropes", [DC, T], f32, kind="ExternalInput").ap()
    mtri = nc.dram_tensor("mtri", [128, 128], f16, kind="ExternalInput").ap()
    ident = nc.dram_tensor("ident", [128, 128], f16, kind="ExternalInput").ap()
    out = nc.dram_tensor("out", [B, T, D], f16, kind="ExternalOutput").ap()

    with tile.TileContext(nc) as tc, ExitStack() as ctx:
        const = ctx.enter_context(tc.tile_pool(name="const", bufs=1))
        x_pool = ctx.enter_context(tc.tile_pool(name="x_pool", bufs=4))
        kt_pool = ctx.enter_context(tc.tile_pool(name="kt_pool", bufs=2))
        va_pool = ctx.enter_context(tc.tile_pool(name="va_pool", bufs=2))
        qt_pool = ctx.enter_context(tc.tile_pool(name="qt_pool", bufs=qt_bufs))
        rope_pool = ctx.enter_context(tc.tile_pool(name="rope_pool", bufs=rope_bufs))
        vt_pool = ctx.enter_context(tc.tile_pool(name="vt_pool", bufs=2))
        p_pool = ctx.enter_context(tc.tile_pool(name="p_pool", bufs=p_bufs))
        nrm_pool = ctx.enter_context(tc.tile_pool(name="nrm_pool", bufs=2))
        ctxn_pool = ctx.enter_context(tc.tile_pool(name="ctxn_pool", bufs=2))
        ost_pool = ctx.enter_context(tc.tile_pool(name="ost_pool", bufs=ost_bufs))

        proj_ps = ctx.enter_context(
            tc.tile_pool(name="proj_ps", bufs=proj_bufs, space=MemorySpace.PSUM)
        )
        s_ps = ctx.enter_context(
            tc.tile_pool(name="s_ps", bufs=s_bufs, space=MemorySpace.PSUM)
        )
        ctx_ps = ctx.enter_context(
            tc.tile_pool(name="ctx_ps", bufs=ctx_bufs, space=MemorySpace.PSUM)
        )
        out_ps = ctx.enter_context(
            tc.tile_pool(name="out_ps", bufs=1, space=MemorySpace.PSUM)
        )

        # ---- constants (first-use order; first window's loads split) ----
        wq_sb = const.tile([128, 8, DC], f16)
        wk_sb = const.tile([128, 8, DC], f16)
        wv_sb = const.tile([128, 8, DC], f16)
        wqr = wq.rearrange("(c p) m -> p c m", p=128)
        xw0 = x_pool.tile([128, 8, QSB], f16, name="xw0", tag="xw")
        xr0 = xT[0, :, 0:QSB].rearrange("(c p) t -> p c t", p=128)
        for kc in range(8):
            nc.sync.dma_start(out=wq_sb[:, kc : kc + 1], in_=wqr[:, kc : kc + 1])
            nc.sync.dma_start(out=xw0[:, kc : kc + 1], in_=xr0[:, kc : kc + 1])
        nc.sync.dma_start(out=wk_sb, in_=wk.rearrange("(c p) m -> p c m", p=128))
        ct_sb = const.tile([128, T], f32)
        st_sb = const.tile([128, T], f32)
        nc.sync.dma_start(out=ct_sb, in_=ropec)
        nc.sync.dma_start(out=st_sb, in_=ropes)
        nc.sync.dma_start(out=wv_sb, in_=wv.rearrange("(c p) m -> p c m", p=128))
        ident_sb = const.tile([128, 128], f16)
        nc.sync.dma_start(out=ident_sb, in_=ident)
        mtri_sb = const.tile([128, 128], f16)
        nc.sync.dma_start(out=mtri_sb, in_=mtri)
        wo_sb = const.tile([128, D], f16)
        nc.sync.dma_start(out=wo_sb, in_=wo)
        # persistent recip staging tile: lanes 1-31 are never read by the
        # zero-mask shuffles but must be initialized once for the checkers
        rc32a = const.tile([32, QSB], f32)
        rc32b = const.tile([32, QSB], f32)
        rc32s = [rc32a, rc32b]
        nc.gpsimd.memset(rc32a, 0.0)
        nc.gpsimd.memset(rc32b, 0.0)

        if True:  # single flattened pipeline across reps
            KTs, VAs = {}, {}
            xws = {0: xw0}

            def issue_xw(seq, j):
                """Emit window j's x DMA early (two windows ahead) so the
                projection matmuls never wait on HBM latency."""
                if j in xws or j >= len(seq):
                    return
                jb, jw = seq[j]
                jt0 = jw * QSB
                xw = x_pool.tile([128, 8, QSB], f16, name="xw", tag="xw")
                nc.sync.dma_start(
                    out=xw,
                    in_=xT[jb, :, jt0 : jt0 + QSB].rearrange(
                        "(c p) t -> p c t", p=128
                    ),
                )
                xws[j] = xw

            def prep_window(i, b, w):
                """Return (qt_tile, quanta closures); xw DMA pre-issued."""
                t0 = w * QSB
                if w == 0:
                    KTs[b] = kt_pool.tile([128, T], f16, name=f"KT{b}", tag="KT")
                    VAs[b] = va_pool.tile(
                        [128, 16, 130], f16, name=f"VA{b}", tag="VA"
                    )
                    nc.gpsimd.memset(VAs[b][:, :, 64:65], 1.0)
                    nc.gpsimd.memset(VAs[b][:, :, 129:130], 1.0)
                KT, VA = KTs[b], VAs[b]
                xw = xws.pop(i)
                qt = qt_pool.tile([128, QSB], f16, name="qt")
                state = {}

                def mk_proj(wsb, key):
                    def run():
                        ps = proj_ps.tile([128, QSB], f32, tag="proj", name="ps")
                        for kc in range(8):
                            nc.tensor.matmul(
                                ps,
                                wsb[:, kc],
                                xw[:, kc],
                                start=(kc == 0),
                                stop=(kc == 7),
                            )
                        state[key] = ps

                    return run

                def mk_rope(key, dest_fn):
                    def run():
                        ps = state.pop(key)
                        dest = dest_fn()
                        sh = rope_pool.tile([128, QSB], f32, tag="ropet", name="sh")
                        nc.vector.stream_shuffle(sh, ps, SWAP_MASK)
                        m1 = rope_pool.tile([128, QSB], f16, tag="ropet", name="m1")
                        nc.vector.tensor_mul(m1, ps, ct_sb[:, t0 : t0 + QSB])
                        m2 = rope_pool.tile([128, QSB], f16, tag="ropet", name="m2")
                        nc.gpsimd.tensor_mul(m2, sh, st_sb[:, t0 : t0 + QSB])
                        nc.gpsimd.tensor_add(dest, m1, m2)

                    return run

                def mk_vpath():
                    def run():
                        ps = state.pop("v")
                        vt = vt_pool.tile([128, QSB], f16, name="vt")
                        nc.vector.tensor_copy(vt, ps)
                        if vxp == "dma":
                            # transpose V via the DMA xbar (SBUF->SBUF): no
                            # PE transposes, no PSUM staging, no engine copies
                            for i in range(4):
                                tb = 4 * w + i
                                nc.sync.dma_start_transpose(
                                    out=VA[:, tb, 0:64],
                                    in_=vt[0:64, 128 * i : 128 * i + 128],
                                )
                                nc.sync.dma_start_transpose(
                                    out=VA[:, tb, 65:129],
                                    in_=vt[64:128, 128 * i : 128 * i + 128],
                                )
                        else:
                            vblk = proj_ps.tile(
                                [128, 4, 128], f16, tag="proj", name="vb"
                            )
                            for i in range(4):
                                nc.tensor.transpose(
                                    vblk[:, i],
                                    vt[:, 128 * i : 128 * i + 128],
                                    ident_sb,
                                )
                            for i in range(4):
                                tb = 4 * w + i
                                fn = (
                                    nc.vector.tensor_copy
                                    if i % 2 == 0
                                    else nc.scalar.copy
                                )
                                fn(VA[:, tb, 0:64], vblk[:, i, 0:64])
                                fn(VA[:, tb, 65:129], vblk[:, i, 64:128])

                    return run

                quanta = [
                    mk_proj(wq_sb, "q"),
                    mk_rope("q", lambda: qt),
                    mk_proj(wk_sb, "k"),
                    mk_rope("k", lambda: KT[:, t0 : t0 + QSB]),
                    mk_proj(wv_sb, "v"),
                    mk_vpath(),
                ]
                return qt, quanta

            def outproj_quanta(ctxn, b, t0):
                """One closure per (ts, eh) out-proj unit, for interleaving.

                The PSUM->SBUF conversion copies alternate DVE/Pool so the
                ACT queue stays a pure exp chain (exp is the attention
                metronome; any ACT copy queued ahead of an exp stalls PV)."""
                qs = []
                for ts in range(4):
                    for eh in range(2):

                        def run(ts=ts, eh=eh):
                            ops = out_ps.tile([128, 512], f32, name="ops", tag="ops")
                            nc.tensor.matmul(
                                ops,
                                ctxn[:, 128 * ts : 128 * ts + 128],
                                wo_sb[:, 512 * eh : 512 * eh + 512],
                                start=True,
                                stop=True,
                            )
                            ost = ost_pool.tile([128, 512], f16, name="ost", tag="ost")
                            nc.vector.tensor_copy(ost, ops)
                            nc.sync.dma_start(
                                out=out[
                                    b,
                                    t0 + 128 * ts : t0 + 128 * ts + 128,
                                    512 * eh : 512 * eh + 512,
                                ],
                                in_=ost,
                            )

                        qs.append(run)
                return qs

            def emit_outproj(ctxn, b, t0):
                for q in outproj_quanta(ctxn, b, t0):
                    q()

            seq = [
                (b, w) for rep in range(reps) for b in range(B) for w in range(NW)
            ]
            qts = {}
            issue_xw(seq, 1)
            qt0, quanta0 = prep_window(0, 0, 0)
            for q in quanta0:
                q()
            qts[0] = qt0
            pending = None

            for i, (b, w) in enumerate(seq):
                t0 = w * QSB
                KT, VA = KTs[b], VAs[b]
                qt = qts.pop(i)
                issue_xw(seq, i + 2)
                if w == NW - 1:
                    issue_xw(seq, i + 3)

                # out-projection of the PREVIOUS window: its deps are long
                # satisfied, so spread its MM+copy units into the interleave
                # (fills out_ps MM->copy->MM serialization with attention work)
                oq = []
                if delay_out and pending is not None:
                    oq = outproj_quanta(*pending)
                    pending = None

                def mix(a, b):
                    """Round-robin merge so the out_ps bank units spread
                    across the whole window instead of clustering early."""
                    m, ia, ib = [], 0, 0
                    while ia < len(a) or ib < len(b):
                        if ia < len(a):
                            m.append(a[ia])
                            ia += 1
                        if ib < len(b):
                            m.append(b[ib])
                            ib += 1
                    return m
                # prefetch: always have window i+1 prepped; during the longest
                # window of each batch (w == NW-1) also prep i+2 so the
                # following SHORT w0 window isn't asked to hide a full prep
                pq = []
                preps = [i + 1, i + 2] if w == NW - 1 else [i + 1]
                for j in preps:
                    if j < len(seq) and j not in qts:
                        jb, jw = seq[j]
                        qtj, pj = prep_window(j, jb, jw)
                        qts[j] = qtj
                        pq += pj
                nq = mix(oq, pq)
                if not pipeline:
                    for q in nq:
                        q()
                    nq = []

                nkb = 4 * w + 4
                cps = [
                    ctx_ps.tile([65, QSB], f32, tag="ctx", name=f"cps{h}")
                    for h in range(2)
                ]
                emitted = 0
                pts = []
                for kb in range(nkb):
                    o = kb - 4 * w
                    col0 = 128 * o if o > 0 else 0
                    ncols = QSB - col0
                    sps = [
                        s_ps.tile([128, QSB], f32, tag="s", name=f"sps{h}")
                        for h in range(2)
                    ]
                    pt = p_pool.tile([128, 2, QSB], f16, tag="pt", name="pt")
                    if o >= 0:
                        # causal mask: seed the diagonal square's PSUM with
                        # -60000 above the diagonal via a PE matmul
                        # (mtri^T @ I), then the QK matmul accumulates onto
                        # it and exp maps those cells to 0. All-PE: no
                        # engine seed, no cross-engine hop, a well-formed
                        # start/stop accumulation group, and the two heads'
                        # seeds share one mtri weight load.
                        for h in range(2):
                            if mask == "mm":
                                nc.tensor.matmul(
                                    sps[h][:, 0:128],
                                    mtri_sb,
                                    ident_sb,
                                    start=True,
                                    stop=False,
                                    skip_group_check=True,
                                )
                            else:
                                nc.scalar.copy(sps[h][:, 0:128], mtri_sb)
                    for h in range(2):
                        if o >= 0:
                            nc.tensor.matmul(
                                sps[h][:, 0:128],
                                KT[64 * h : 64 * h + 64, 128 * kb : 128 * kb + 128],
                                qt[64 * h : 64 * h + 64, col0 : col0 + 128],
                                start=False,
                                stop=True,
                                skip_group_check=True,
                            )
                            if ncols > 128:
                                nc.tensor.matmul(
                                    sps[h][:, 128:ncols],
                                    KT[
                                        64 * h : 64 * h + 64,
                                        128 * kb : 128 * kb + 128,
                                    ],
                                    qt[64 * h : 64 * h + 64, col0 + 128 : QSB],
                                    start=True,
                                    stop=True,
                                    skip_group_check=True,
                                )
                        else:
                            nc.tensor.matmul(
                                sps[h][:, :ncols],
                                KT[64 * h : 64 * h + 64, 128 * kb : 128 * kb + 128],
                                qt[64 * h : 64 * h + 64, col0:QSB],
                                start=True,
                                stop=True,
                            )
                        # per-head exp right behind its QK: exp(h0) runs on
                        # ACT while QK(h1) still streams, and PV(h0) starts
                        # as soon as exp(h0) lands
                        nc.scalar.activation(
                            pt[:, h, :ncols],
                            sps[h][:, :ncols],
                            AF.Exp,
                            scale=0.125,
                        )
                    # interleave next-window projection quanta while ACT runs
                    # exp, holding back `rsv` of them for after normalize
                    want = (kb + 1) * max(len(nq) - rsv, 0) // nkb
                    while emitted < want:
                        nq[emitted]()
                        emitted += 1
                    # PV emission runs `skew` kbs behind QK so PE never
                    # head-of-line blocks on exp(kb) or on the previous
                    # window's ctx-bank release — there is always a later
                    # QK already queued behind the PV's wait
                    pts.append((pt, col0, ncols))
                    pkb = kb - skew
                    if pkb >= 0:
                        ppt, pcol0, pncols = pts[pkb]
                        for h in range(2):
                            nc.tensor.matmul(
                                cps[h][:, pcol0:QSB],
                                VA[:, pkb, 65 * h : 65 * h + 65],
                                ppt[:, h, :pncols],
                                start=(pkb == 0),
                                stop=(pkb == nkb - 1),
                            )
                for pkb in range(max(nkb - skew, 0), nkb):
                    ppt, pcol0, pncols = pts[pkb]
                    for h in range(2):
                        nc.tensor.matmul(
                            cps[h][:, pcol0:QSB],
                            VA[:, pkb, 65 * h : 65 * h + 65],
                            ppt[:, h, :pncols],
                            start=(pkb == 0),
                            stop=(pkb == nkb - 1),
                        )
                # flush all but a small reserve of quanta; the reserve is
                # emitted after normalize so PE has work queued while the
                # recip/broadcast/mul chain frees the ctx PSUM banks
                reserve = min(rsv, len(nq) - emitted)
                while emitted < len(nq) - reserve:
                    nq[emitted]()
                    emitted += 1

                # normalize this window (emitted before the reserve so DVE
                # starts the recip as soon as the last PV lands). Phase
                # order — both recips, both broadcasts, both muls — so
                # recip(h1) isn't queued behind mul(h0)'s wait on the Pool
                # broadcast, and both ctx banks release as early as possible.
                ctxn = ctxn_pool.tile([128, QSB], f16, name="ctxn")
                if nrm == "copy":  # timing probe only: skip the divide
                    for h in range(2):
                        nc.vector.tensor_copy(
                            ctxn[64 * h : 64 * h + 64, :], cps[h][0:64, :]
                        )
                elif nrm == "dve":
                    # all-DVE chain (recip into lane 0 of the persistent
                    # staging tile, zero-mask shuffles broadcast to both
                    # quadrants): no Pool hop, so the chain can't queue
                    # behind the rope tails that pile up on Pool at window
                    # end. Phased h0/h1 so both ctx banks release early.
                    bcs = []
                    for h in range(2):
                        bcs.append(nrm_pool.tile([64, QSB], f32, tag="bc", name="bc"))
                        nc.vector.reciprocal(rc32s[h][0:1, :], cps[h][64:65, :])
                    for h in range(2):
                        nc.vector.stream_shuffle(bcs[h][0:32, :], rc32s[h], [0] * 32)
                        nc.vector.stream_shuffle(bcs[h][32:64, :], rc32s[h], [0] * 32)
                    for h in range(2):
                        # DVE bank routing at nch=64 can write either
                        # half-plane, so head1's product lands at partitions
                        # 64-127 directly
                        nc.vector.tensor_mul(
                            ctxn[64 * h : 64 * h + 64, :], cps[h][0:64, :], bcs[h]
                        )
                else:
                    bcs, rcs = [], []
                    for h in range(2):
                        bcs.append(nrm_pool.tile([64, QSB], f32, tag="bc", name="bc"))
                        rcs.append(
                            nrm_pool.tile([1, QSB], f32, tag="rc0", name="rc0")
                        )
                        nc.vector.reciprocal(rcs[h], cps[h][64:65, :])
                    for h in range(2):
                        nc.gpsimd.partition_broadcast(bcs[h], rcs[h], channels=64)
                    for h in range(2):
                        # DVE bank routing at nch=64 can write either
                        # half-plane, so head1's product lands at partitions
                        # 64-127 directly
                        nc.vector.tensor_mul(
                            ctxn[64 * h : 64 * h + 64, :], cps[h][0:64, :], bcs[h]
                        )
                while emitted < len(nq):
                    nq[emitted]()
                    emitted += 1
                if delay_out:
                    pending = (ctxn, b, t0)
                else:
                    emit_outproj(ctxn, b, t0)

            if pending is not None:
                emit_outproj(*pending)

    nc.compile()
    return nc


def _host_inputs(x, Wq, Wk, Wv, Wo):
    xT = np.ascontiguousarray(x.transpose(0, 2, 1)).astype(np.float16)

    pos = np.arange(T, dtype=np.float64)
    inv_freq = np.power(10000.0, -2.0 * np.arange(0, DH, 2) / DH)  # (32,)
    freqs = pos[:, None] * inv_freq[None, :]  # (T, 32)
    cos = np.cos(freqs)
    sin = np.sin(freqs)
    ct = np.empty((DC, T), np.float32)
    st = np.empty((DC, T), np.float32)
    for p in range(DC):
        i = (p % DH) // 2
        ct[p] = cos[:, i]
        st[p] = sin[:, i] * (-1.0 if p % 2 == 0 else 1.0)

    pp, cc = np.meshgrid(np.arange(128), np.arange(128), indexing="ij")
    # mtri[q, k] = -60000 where k > q: contracted with the identity it
    # seeds the causal mask into the diagonal score square (f16-safe value;
    # after the 0.125 exp scale it still flushes exp to exactly 0)
    mtri = np.where(cc > pp, -60000.0, 0.0).astype(np.float16)
    ident = np.eye(128, dtype=np.float32)

    per_core = []
    for c in range(NCORES):
        sl = slice(c * DC, (c + 1) * DC)
        per_core.append(
            {
                "xt": xT,
                "wq": np.ascontiguousarray(Wq[:, sl]).astype(np.float16),
                "wk": np.ascontiguousarray(Wk[:, sl]).astype(np.float16),
                "wv": np.ascontiguousarray(Wv[:, sl]).astype(np.float16),
                "wo": np.ascontiguousarray(Wo[sl, :]).astype(np.float16),
                "ropec": ct,
                "ropes": st,
                "mtri": mtri,
                "ident": ident.astype(np.float16),
            }
        )
    return per_core


def kernel(x, Wq, Wk, Wv, Wo, bo):
    x = np.asarray(x, np.float32)
    Wq = np.asarray(Wq, np.float32)
    Wk = np.asarray(Wk, np.float32)
    Wv = np.asarray(Wv, np.float32)
    Wo = np.asarray(Wo, np.float32)
    bo = np.asarray(bo, np.float32)

    if "nc" not in _CACHE:
        _CACHE["nc"] = _build()
    nc = _CACHE["nc"]

    in_maps = _host_inputs(x, Wq, Wk, Wv, Wo)
    res = run_bass_kernel_spmd(nc, in_maps, list(range(NCORES)))
    acc = res.results[0]["out"].astype(np.float64)
    for c in range(1, NCORES):
        acc += res.results[c]["out"]
    acc += bo.astype(np.float64)
    return acc.astype(np.float32)



# revision 59
# speedup vs baseline: 1.1716x; 1.0915x over previous
"""Multi-head causal attention with RoPE on 8 Trainium2 NeuronCores.

Sharding: tensor-parallel over heads. Each core owns 2 of the 16 heads
(a 128-wide slice of D_OUT): it computes Q/K/V projections for its slice
(column-sliced Wq/Wk/Wv), RoPE, causal attention, and a row-sliced
out-projection partial. The 8 partials are summed on the host (the
all-reduce of the row-parallel out projection) and the bias added once.

All matmul operands are fp16 (weights/x quantized on the host, Q/K/V/P
requantized on-chip); PSUM accumulation stays fp32, so end-to-end error
is ~4e-4. fp16 matters on real HW: f32r matmuls use a slow fused 4-byte
self-loading weight path (~4x slower than the cost model's optimistic
rate), while fp16 streams 1 col/cycle with overlapped FWL weight loads.

Scheduling: one flattened software pipeline across all reps/batches/
windows for the in-order engine queues. Per key-block the two heads'
scores live in one 2-bank PSUM tile so a single wide exp serves both
(halves ACT instruction overhead). The next window's projections, its
RoPE, and the PREVIOUS window's out-projection units are interleaved
into the attention loop, so PE never sits behind the ACT exp chain or
the normalize chain. During each batch's last (longest) window the
pipeline preps two windows ahead so the short w0 window isn't asked to
hide a full projection. Normalize is a pure-DVE chain (recip with
cross-partition addressing, zero-mask stream_shuffle quadrant
broadcasts, half-plane product writes) so no cross-engine semaphore
hop sits between the last PV and releasing the ctx PSUM banks; a small
reserve of interleave quanta is flushed after it so PE stays fed while
the chain drains.
"""

import sys

sys.path.insert(0, "/opt/trn_rl_repo")

from contextlib import ExitStack

import numpy as np

import concourse.bass as bass
import concourse.tile as tile
from concourse import bacc, mybir
from concourse.bass import MemorySpace
from concourse.bass_utils import run_bass_kernel_spmd

B, T, D, H, DH = 2, 2048, 1024, 16, 64
NCORES = 8
DC = D // NCORES  # 128: d-slice per core (2 heads)
QSB = 512  # query superblock
NW = T // QSB  # windows per batch

f32 = mybir.dt.float32
f32r = mybir.dt.float32r
f16 = mybir.dt.float16
AF = mybir.ActivationFunctionType

SWAP_MASK = []
for _i in range(16):
    SWAP_MASK += [2 * _i + 1, 2 * _i]

_CACHE = {}


def _build(reps=1, pipeline=True, delay_out=True, merge_out=False, cp="split", nrm="gps", qt_bufs=3, rope_bufs=12, s_bufs=3, ctx_bufs=2, ost_bufs=8, p_bufs=6, proj_bufs=2, rsv=6, skew=0, vxp="pe", mask="mm"):
    nc = bacc.Bacc("TRN2", target_bir_lowering=False, debug=False)
    xT = nc.dram_tensor("xt", [B, D, T], f16, kind="ExternalInput").ap()
    wq = nc.dram_tensor("wq", [D, DC], f16, kind="ExternalInput").ap()
    wk = nc.dram_tensor("wk", [D, DC], f16, kind="ExternalInput").ap()
    wv = nc.dram_tensor("wv", [D, DC], f16, kind="ExternalInput").ap()
    wo = nc.dram_tensor("wo", [DC, D], f16, kind="ExternalInput").ap()
    ropec = nc.dram_tensor("ropec", [DC, T], f32, kind="ExternalInput").ap()
    ropes = nc.dram_tensor("ropes", [DC, T], f32, kind="ExternalInput").ap()
    mtri = nc.dram_tensor("mtri", [128, 128], f16, kind="ExternalInput").ap()
    ident = nc.dram_tensor("ident", [128, 128], f16, kind="ExternalInput").ap()
    out = nc.dram_tensor("out", [B, T, D], f16, kind="ExternalOutput").ap()

    with tile.TileContext(nc) as tc, ExitStack() as ctx:
        const = ctx.enter_context(tc.tile_pool(name="const", bufs=1))
        x_pool = ctx.enter_context(tc.tile_pool(name="x_pool", bufs=4))
        kt_pool = ctx.enter_context(tc.tile_pool(name="kt_pool", bufs=2))
        va_pool = ctx.enter_context(tc.tile_pool(name="va_pool", bufs=2))
        qt_pool = ctx.enter_context(tc.tile_pool(name="qt_pool", bufs=qt_bufs))
        rope_pool = ctx.enter_context(tc.tile_pool(name="rope_pool", bufs=rope_bufs))
        vt_pool = ctx.enter_context(tc.tile_pool(name="vt_pool", bufs=2))
        p_pool = ctx.enter_context(tc.tile_pool(name="p_pool", bufs=p_bufs))
        nrm_pool = ctx.enter_context(tc.tile_pool(name="nrm_pool", bufs=2))
        ctxn_pool = ctx.enter_context(tc.tile_pool(name="ctxn_pool", bufs=2))
        ost_pool = ctx.enter_context(tc.tile_pool(name="ost_pool", bufs=ost_bufs))

        proj_ps = ctx.enter_context(
            tc.tile_pool(name="proj_ps", bufs=proj_bufs, space=MemorySpace.PSUM)
        )
        s_ps = ctx.enter_context(
            tc.tile_pool(name="s_ps", bufs=s_bufs, space=MemorySpace.PSUM)
        )
        ctx_ps = ctx.enter_context(
            tc.tile_pool(name="ctx_ps", bufs=ctx_bufs, space=MemorySpace.PSUM)
        )
        out_ps = ctx.enter_context(
            tc.tile_pool(name="out_ps", bufs=1, space=MemorySpace.PSUM)
        )

        # ---- constants (first-use order; first window's loads split) ----
        wq_sb = const.tile([128, 8, DC], f16)
        wk_sb = const.tile([128, 8, DC], f16)
        wv_sb = const.tile([128, 8, DC], f16)
        wqr = wq.rearrange("(c p) m -> p c m", p=128)
        xw0 = x_pool.tile([128, 8, QSB], f16, name="xw0", tag="xw")
        xr0 = xT[0, :, 0:QSB].rearrange("(c p) t -> p c t", p=128)
        for kc in range(8):
            nc.sync.dma_start(out=wq_sb[:, kc : kc + 1], in_=wqr[:, kc : kc + 1])
            nc.sync.dma_start(out=xw0[:, kc : kc + 1], in_=xr0[:, kc : kc + 1])
        nc.sync.dma_start(out=wk_sb, in_=wk.rearrange("(c p) m -> p c m", p=128))
        ct_sb = const.tile([128, T], f32)
        st_sb = const.tile([128, T], f32)
        nc.sync.dma_start(out=ct_sb, in_=ropec)
        nc.sync.dma_start(out=st_sb, in_=ropes)
        nc.sync.dma_start(out=wv_sb, in_=wv.rearrange("(c p) m -> p c m", p=128))
        ident_sb = const.tile([128, 128], f16)
        nc.sync.dma_start(out=ident_sb, in_=ident)
        mtri_sb = const.tile([128, 128], f16)
        nc.sync.dma_start(out=mtri_sb, in_=mtri)
        wo_sb = const.tile([128, D], f16)
        nc.sync.dma_start(out=wo_sb, in_=wo)
        # persistent recip staging tile: lanes 1-31 are never read by the
        # zero-mask shuffles but must be initialized once for the checkers
        rc32a = const.tile([32, QSB], f32)
        rc32b = const.tile([32, QSB], f32)
        rc32s = [rc32a, rc32b]
        nc.gpsimd.memset(rc32a, 0.0)
        nc.gpsimd.memset(rc32b, 0.0)

        if True:  # single flattened pipeline across reps
            KTs, VAs = {}, {}
            xws = {0: xw0}

            def issue_xw(seq, j):
                """Emit window j's x DMA early (two windows ahead) so the
                projection matmuls never wait on HBM latency."""
                if j in xws or j >= len(seq):
                    return
                jb, jw = seq[j]
                jt0 = jw * QSB
                xw = x_pool.tile([128, 8, QSB], f16, name="xw", tag="xw")
                nc.sync.dma_start(
                    out=xw,
                    in_=xT[jb, :, jt0 : jt0 + QSB].rearrange(
                        "(c p) t -> p c t", p=128
                    ),
                )
                xws[j] = xw

            def prep_window(i, b, w):
                """Return (qt_tile, quanta closures); xw DMA pre-issued."""
                t0 = w * QSB
                if w == 0:
                    KTs[b] = kt_pool.tile([128, T], f16, name=f"KT{b}", tag="KT")
                    VAs[b] = va_pool.tile(
                        [128, 16, 130], f16, name=f"VA{b}", tag="VA"
                    )
                    nc.gpsimd.memset(VAs[b][:, :, 64:65], 1.0)
                    nc.gpsimd.memset(VAs[b][:, :, 129:130], 1.0)
                KT, VA = KTs[b], VAs[b]
                xw = xws.pop(i)
                qt = qt_pool.tile([128, QSB], f16, name="qt")
                state = {}

                def mk_proj(wsb, key):
                    def run():
                        ps = proj_ps.tile([128, QSB], f32, tag="proj", name="ps")
                        for kc in range(8):
                            nc.tensor.matmul(
                                ps,
                                wsb[:, kc],
                                xw[:, kc],
                                start=(kc == 0),
                                stop=(kc == 7),
                            )
                        state[key] = ps

                    return run

                def mk_rope(key, dest_fn):
                    def run():
                        ps = state.pop(key)
                        dest = dest_fn()
                        sh = rope_pool.tile([128, QSB], f32, tag="ropet", name="sh")
                        nc.vector.stream_shuffle(sh, ps, SWAP_MASK)
                        m1 = rope_pool.tile([128, QSB], f16, tag="ropet", name="m1")
                        nc.vector.tensor_mul(m1, ps, ct_sb[:, t0 : t0 + QSB])
                        m2 = rope_pool.tile([128, QSB], f16, tag="ropet", name="m2")
                        nc.gpsimd.tensor_mul(m2, sh, st_sb[:, t0 : t0 + QSB])
                        nc.gpsimd.tensor_add(dest, m1, m2)

                    return run

                def mk_vpath():
                    def run():
                        ps = state.pop("v")
                        vt = vt_pool.tile([128, QSB], f16, name="vt")
                        nc.vector.tensor_copy(vt, ps)
                        if vxp == "dma":
                            # transpose V via the DMA xbar (SBUF->SBUF): no
                            # PE transposes, no PSUM staging, no engine copies
                            for i in range(4):
                                tb = 4 * w + i
                                nc.sync.dma_start_transpose(
                                    out=VA[:, tb, 0:64],
                                    in_=vt[0:64, 128 * i : 128 * i + 128],
                                )
                                nc.sync.dma_start_transpose(
                                    out=VA[:, tb, 65:129],
                                    in_=vt[64:128, 128 * i : 128 * i + 128],
                                )
                        else:
                            vblk = proj_ps.tile(
                                [128, 4, 128], f16, tag="proj", name="vb"
                            )
                            for i in range(4):
                                nc.tensor.transpose(
                                    vblk[:, i],
                                    vt[:, 128 * i : 128 * i + 128],
                                    ident_sb,
                                )
                            for i in range(4):
                                tb = 4 * w + i
                                fn = (
                                    nc.vector.tensor_copy
                                    if i % 2 == 0
                                    else nc.scalar.copy
                                )
                                fn(VA[:, tb, 0:64], vblk[:, i, 0:64])
                                fn(VA[:, tb, 65:129], vblk[:, i, 64:128])

                    return run

                quanta = [
                    mk_proj(wq_sb, "q"),
                    mk_rope("q", lambda: qt),
                    mk_proj(wk_sb, "k"),
                    mk_rope("k", lambda: KT[:, t0 : t0 + QSB]),
                    mk_proj(wv_sb, "v"),
                    mk_vpath(),
                ]
                return qt, quanta

            def outproj_quanta(ctxn, b, t0):
                """One closure per (ts, eh) out-proj unit, for interleaving.

                The PSUM->SBUF conversion copies alternate DVE/Pool so the
                ACT queue stays a pure exp chain (exp is the attention
                metronome; any ACT copy queued ahead of an exp stalls PV)."""
                qs = []
                for ts in range(4):
                    for eh in range(2):

                        def run(ts=ts, eh=eh):
                            ops = out_ps.tile([128, 512], f32, name="ops", tag="ops")
                            nc.tensor.matmul(
                                ops,
                                ctxn[:, 128 * ts : 128 * ts + 128],
                                wo_sb[:, 512 * eh : 512 * eh + 512],
                                start=True,
                                stop=True,
                            )
                            ost = ost_pool.tile([128, 512], f16, name="ost", tag="ost")
                            nc.vector.tensor_copy(ost, ops)
                            nc.sync.dma_start(
                                out=out[
                                    b,
                                    t0 + 128 * ts : t0 + 128 * ts + 128,
                                    512 * eh : 512 * eh + 512,
                                ],
                                in_=ost,
                            )

                        qs.append(run)
                return qs

            def emit_outproj(ctxn, b, t0):
                for q in outproj_quanta(ctxn, b, t0):
                    q()

            seq = [
                (b, w) for rep in range(reps) for b in range(B) for w in range(NW)
            ]
            qts = {}
            issue_xw(seq, 1)
            qt0, quanta0 = prep_window(0, 0, 0)
            for q in quanta0:
                q()
            qts[0] = qt0
            pending = None

            for i, (b, w) in enumerate(seq):
                t0 = w * QSB
                KT, VA = KTs[b], VAs[b]
                qt = qts.pop(i)
                issue_xw(seq, i + 2)
                if w == NW - 1:
                    issue_xw(seq, i + 3)

                # out-projection of the PREVIOUS window: its deps are long
                # satisfied, so spread its MM+copy units into the interleave
                # (fills out_ps MM->copy->MM serialization with attention work)
                oq = []
                if delay_out and pending is not None:
                    oq = outproj_quanta(*pending)
                    pending = None

                def mix(a, b):
                    """Round-robin merge so the out_ps bank units spread
                    across the whole window instead of clustering early."""
                    m, ia, ib = [], 0, 0
                    while ia < len(a) or ib < len(b):
                        if ia < len(a):
                            m.append(a[ia])
                            ia += 1
                        if ib < len(b):
                            m.append(b[ib])
                            ib += 1
                    return m
                # prefetch: always have window i+1 prepped; during the longest
                # window of each batch (w == NW-1) also prep i+2 so the
                # following SHORT w0 window isn't asked to hide a full prep
                pq = []
                preps = [i + 1, i + 2] if w == NW - 1 else [i + 1]
                for j in preps:
                    if j < len(seq) and j not in qts:
                        jb, jw = seq[j]
                        qtj, pj = prep_window(j, jb, jw)
                        qts[j] = qtj
                        pq += pj
                nq = mix(oq, pq)
                if not pipeline:
                    for q in nq:
                        q()
                    nq = []

                nkb = 4 * w + 4
                cps = [
                    ctx_ps.tile([65, QSB], f32, tag="ctx", name=f"cps{h}")
                    for h in range(2)
                ]
                emitted = 0
                pts = []
                for kb in range(nkb):
                    o = kb - 4 * w
                    col0 = 128 * o if o > 0 else 0
                    ncols = QSB - col0
                    sps = [
                        s_ps.tile([128, QSB], f32, tag="s", name=f"sps{h}")
                        for h in range(2)
                    ]
                    pt = p_pool.tile([128, 2, QSB], f16, tag="pt", name="pt")
                    for h in range(2):
                        if o >= 0:
                            # causal mask: seed the diagonal square's PSUM
                            # with -60000 above the diagonal via a PE matmul
                            # (mtri^T @ I), then the QK matmul accumulates
                            # onto it and exp maps those cells to 0. All-PE:
                            # no engine seed, no cross-engine hop, and a
                            # well-formed start/stop accumulation group.
                            # Per-head emission: h1's seed waits its s-buffer
                            # (exp of an earlier block) and must not sit
                            # ahead of h0's QK in the PE queue.
                            if mask == "mm":
                                nc.tensor.matmul(
                                    sps[h][:, 0:128],
                                    mtri_sb,
                                    ident_sb,
                                    start=True,
                                    stop=False,
                                    skip_group_check=True,
                                )
                            else:
                                nc.scalar.copy(sps[h][:, 0:128], mtri_sb)
                            nc.tensor.matmul(
                                sps[h][:, 0:128],
                                KT[64 * h : 64 * h + 64, 128 * kb : 128 * kb + 128],
                                qt[64 * h : 64 * h + 64, col0 : col0 + 128],
                                start=False,
                                stop=True,
                                skip_group_check=True,
                            )
                            if ncols > 128:
                                nc.tensor.matmul(
                                    sps[h][:, 128:ncols],
                                    KT[
                                        64 * h : 64 * h + 64,
                                        128 * kb : 128 * kb + 128,
                                    ],
                                    qt[64 * h : 64 * h + 64, col0 + 128 : QSB],
                                    start=True,
                                    stop=True,
                                    skip_group_check=True,
                                )
                        else:
                            nc.tensor.matmul(
                                sps[h][:, :ncols],
                                KT[64 * h : 64 * h + 64, 128 * kb : 128 * kb + 128],
                                qt[64 * h : 64 * h + 64, col0:QSB],
                                start=True,
                                stop=True,
                            )
                        # per-head exp right behind its QK: exp(h0) runs on
                        # ACT while QK(h1) still streams, and PV(h0) starts
                        # as soon as exp(h0) lands
                        nc.scalar.activation(
                            pt[:, h, :ncols],
                            sps[h][:, :ncols],
                            AF.Exp,
                            scale=0.125,
                        )
                    # interleave next-window projection quanta while ACT runs
                    # exp, holding back `rsv` of them for after normalize
                    want = (kb + 1) * max(len(nq) - rsv, 0) // nkb
                    while emitted < want:
                        nq[emitted]()
                        emitted += 1
                    # PV emission runs `skew` kbs behind QK so PE never
                    # head-of-line blocks on exp(kb) or on the previous
                    # window's ctx-bank release — there is always a later
                    # QK already queued behind the PV's wait
                    pts.append((pt, col0, ncols))
                    pkb = kb - skew
                    if pkb >= 0:
                        ppt, pcol0, pncols = pts[pkb]
                        for h in range(2):
                            nc.tensor.matmul(
                                cps[h][:, pcol0:QSB],
                                VA[:, pkb, 65 * h : 65 * h + 65],
                                ppt[:, h, :pncols],
                                start=(pkb == 0),
                                stop=(pkb == nkb - 1),
                            )
                for pkb in range(max(nkb - skew, 0), nkb):
                    ppt, pcol0, pncols = pts[pkb]
                    for h in range(2):
                        nc.tensor.matmul(
                            cps[h][:, pcol0:QSB],
                            VA[:, pkb, 65 * h : 65 * h + 65],
                            ppt[:, h, :pncols],
                            start=(pkb == 0),
                            stop=(pkb == nkb - 1),
                        )
                # flush all but a small reserve of quanta; the reserve is
                # emitted after normalize so PE has work queued while the
                # recip/broadcast/mul chain frees the ctx PSUM banks
                reserve = min(rsv, len(nq) - emitted)
                while emitted < len(nq) - reserve:
                    nq[emitted]()
                    emitted += 1

                # normalize this window (emitted before the reserve so DVE
                # starts the recip as soon as the last PV lands). Phase
                # order — both recips, both broadcasts, both muls — so
                # recip(h1) isn't queued behind mul(h0)'s wait on the Pool
                # broadcast, and both ctx banks release as early as possible.
                ctxn = ctxn_pool.tile([128, QSB], f16, name="ctxn")
                if nrm == "copy":  # timing probe only: skip the divide
                    for h in range(2):
                        nc.vector.tensor_copy(
                            ctxn[64 * h : 64 * h + 64, :], cps[h][0:64, :]
                        )
                elif nrm == "dve":
                    # all-DVE chain (recip into lane 0 of the persistent
                    # staging tile, zero-mask shuffles broadcast to both
                    # quadrants): no Pool hop, so the chain can't queue
                    # behind the rope tails that pile up on Pool at window
                    # end. Phased h0/h1 so both ctx banks release early.
                    bcs = []
                    for h in range(2):
                        bcs.append(nrm_pool.tile([64, QSB], f32, tag="bc", name="bc"))
                        nc.vector.reciprocal(rc32s[h][0:1, :], cps[h][64:65, :])
                    for h in range(2):
                        nc.vector.stream_shuffle(bcs[h][0:32, :], rc32s[h], [0] * 32)
                        nc.vector.stream_shuffle(bcs[h][32:64, :], rc32s[h], [0] * 32)
                    for h in range(2):
                        # DVE bank routing at nch=64 can write either
                        # half-plane, so head1's product lands at partitions
                        # 64-127 directly
                        nc.vector.tensor_mul(
                            ctxn[64 * h : 64 * h + 64, :], cps[h][0:64, :], bcs[h]
                        )
                else:
                    bcs, rcs = [], []
                    for h in range(2):
                        bcs.append(nrm_pool.tile([64, QSB], f32, tag="bc", name="bc"))
                        rcs.append(
                            nrm_pool.tile([1, QSB], f32, tag="rc0", name="rc0")
                        )
                        nc.vector.reciprocal(rcs[h], cps[h][64:65, :])
                    for h in range(2):
                        nc.gpsimd.partition_broadcast(bcs[h], rcs[h], channels=64)
                    for h in range(2):
                        # DVE bank routing at nch=64 can write either
                        # half-plane, so head1's product lands at partitions
                        # 64-127 directly
                        nc.vector.tensor_mul(
                            ctxn[64 * h : 64 * h + 64, :], cps[h][0:64, :], bcs[h]
                        )
                while emitted < len(nq):
                    nq[emitted]()
                    emitted += 1
                if delay_out:
                    pending = (ctxn, b, t0)
                else:
                    emit_outproj(ctxn, b, t0)

            if pending is not None:
                emit_outproj(*pending)

    nc.compile()
    return nc


def _host_inputs(x, Wq, Wk, Wv, Wo):
    xT = np.ascontiguousarray(x.transpose(0, 2, 1)).astype(np.float16)

    pos = np.arange(T, dtype=np.float64)
    inv_freq = np.power(10000.0, -2.0 * np.arange(0, DH, 2) / DH)  # (32,)
    freqs = pos[:, None] * inv_freq[None, :]  # (T, 32)
    cos = np.cos(freqs)
    sin = np.sin(freqs)
    ct = np.empty((DC, T), np.float32)
    st = np.empty((DC, T), np.float32)
    for p in range(DC):
        i = (p % DH) // 2
        ct[p] = cos[:, i]
        st[p] = sin[:, i] * (-1.0 if p % 2 == 0 else 1.0)

    pp, cc = np.meshgrid(np.arange(128), np.arange(128), indexing="ij")
    # mtri[q, k] = -60000 where k > q: contracted with the identity it
    # seeds the causal mask into the diagonal score square (f16-safe value;
    # after the 0.125 exp scale it still flushes exp to exactly 0)
    mtri = np.where(cc > pp, -60000.0, 0.0).astype(np.float16)
    ident = np.eye(128, dtype=np.float32)

    per_core = []
    for c in range(NCORES):
        sl = slice(c * DC, (c + 1) * DC)
        per_core.append(
            {
                "xt": xT,
                "wq": np.ascontiguousarray(Wq[:, sl]).astype(np.float16),
                "wk": np.ascontiguousarray(Wk[:, sl]).astype(np.float16),
                "wv": np.ascontiguousarray(Wv[:, sl]).astype(np.float16),
                "wo": np.ascontiguousarray(Wo[sl, :]).astype(np.float16),
                "ropec": ct,
                "ropes": st,
                "mtri": mtri,
                "ident": ident.astype(np.float16),
            }
        )
    return per_core


def kernel(x, Wq, Wk, Wv, Wo, bo):
    x = np.asarray(x, np.float32)
    Wq = np.asarray(Wq, np.float32)
    Wk = np.asarray(Wk, np.float32)
    Wv = np.asarray(Wv, np.float32)
    Wo = np.asarray(Wo, np.float32)
    bo = np.asarray(bo, np.float32)

    if "nc" not in _CACHE:
        _CACHE["nc"] = _build()
    nc = _CACHE["nc"]

    in_maps = _host_inputs(x, Wq, Wk, Wv, Wo)
    res = run_bass_kernel_spmd(nc, in_maps, list(range(NCORES)))
    acc = res.results[0]["out"].astype(np.float64)
    for c in range(1, NCORES):
        acc += res.results[c]["out"]
    acc += bo.astype(np.float64)
    return acc.astype(np.float32)

